# revision 1
# baseline (speedup 1.0000x reference)
"""Trainium2 Bass kernel for nn_AttentionBlock (GN + self-attn + cross-attn + FFN).

Sharding: data-parallel over batch B=8 -> one batch element per NeuronCore.
Per-core layout: activations as [C(partitions), L(free)] "conv" layout.
Attention computed with transposed scores S^T[m, l]; softmax sums come from an
augmented-V matmul (extra ones column -> Z lands in psum partition 64), so no
cross-partition reductions or transposes are needed. Row-softmax max-subtraction
is skipped (logits are provably < 2 for this block's scale).
Matmuls run in float32r (rounded fp32, full PE speed at N>=256); attention
probabilities / V / cross-attention / FFN-hidden run in bf16.
"""
import sys

for _p in ("/opt/trn_rl_repo", "/root/.axon_site/_ro/trn_rl_repo"):
    if _p not in sys.path:
        sys.path.append(_p)

import numpy as np

# ---- problem constants (hardcoded per contract) ----
B, C, H, W = 8, 512, 32, 32
L = H * W                       # 1024
NH, HD = 8, 64
CT = C // 128                   # 4 channel tiles
LT = L // 128                   # 8 l/m tiles
NCH = 2                         # l chunks of 512
CH = L // NCH                   # 512
CTX = 768
S = 77
SP = 128                        # padded context tokens
KTC = CTX // 128                # 6
FF = 4 * C                      # 2048
FT = FF // 128                  # 16
G = 32                          # groups
EPS = 1e-5
SCALE = HD ** -0.5

_CACHE = {}


def _build(gelu_identity=False, stop_after=None, repeat=1, gn2_skip=False):
    import concourse.mybir as mybir
    import concourse.tile as tile
    from concourse import bacc

    f32 = mybir.dt.float32
    f32r = mybir.dt.float32r
    bf16 = mybir.dt.bfloat16
    Exp = mybir.ActivationFunctionType.Exp
    Gelu = (mybir.ActivationFunctionType.Identity if gelu_identity
            else mybir.ActivationFunctionType.Gelu)
    Sqrt = mybir.ActivationFunctionType.Sqrt
    Square = mybir.ActivationFunctionType.Square
    add = mybir.AluOpType.add
    mult = mybir.AluOpType.mult
    AX = mybir.AxisListType.X

    nc = bacc.Bacc("TRN2", target_bir_lowering=False, debug=False, num_devices=8)

    def din(name, shape, dt=f32r):
        return nc.dram_tensor(name, shape, dt, kind="ExternalInput").ap()

    x_d = din("x", [128, CT, L], f32)
    ctxT_d = din("ctxT", [128, KTC, SP], bf16)
    qkvwT_d = din("qkv_wT", [128, CT, 3 * C])
    sapT_d = din("sa_proj_wT", [128, CT, C], bf16)
    qwT_d = din("q_wT", [128, CT, C])
    kwT_d = din("k_wT", [128, KTC, C], bf16)
    vwT_d = din("v_wT", [128, KTC, C], bf16)
    capT_d = din("ca_proj_wT", [128, CT, C], bf16)
    w1T_d = din("w1T", [128, CT, FF])
    w2T_d = din("w2T", [128, FT, C], bf16)
    mask_d = din("gn_mask", [128, CT, G], f32)
    maskT_d = din("gn_maskT", [G, C], f32)
    gn1g_d = din("gn1g", [128, CT], f32)
    gn1b_d = din("gn1b", [128, CT], f32)
    gn2g_d = din("gn2g", [128, CT], f32)
    gn2b_d = din("gn2b", [128, CT], f32)
    qkb_d = din("qkb", [128, 2 * CT], f32)     # qkv_b for q,k in conv layout
    vb_row_d = din("vb_row", [1, C], f32)      # qkv_b v-part as a row
    sapb_d = din("sapb", [128, CT], f32)
    qb_d = din("qb", [128, CT], f32)
    kb_d = din("kb", [128, CT], f32)
    vb2_row_d = din("vb2_row", [1, C], f32)
    capb_d = din("capb", [128, CT], f32)
    b1_d = din("b1", [128, FT], f32)
    b2_d = din("b2", [128, CT], f32)
    smask_d = din("smask", [128, 1], f32)      # context token validity column

    out_d = nc.dram_tensor("out", [128, CT, L], f32, kind="ExternalOutput").ap()

    dma = nc.sync.dma_start

    class _Stop(Exception):
        pass

    with tile.TileContext(nc) as tc:
        _stack = []

        def apool(**kw):
            p = tc.alloc_tile_pool(**kw)
            _stack.append(p)
            return p

        def rel(p):
            assert _stack[-1] is p
            _stack.pop()
            p.release()

        _base_depth = [0]

        def stop_dump(src):
            """Truncated build: dump src, unwind pools opened within this pass."""
            for ct in range(CT):
                dma(out=out_d[:, ct, :], in_=src[:, ct, :].bitcast(f32))
            while len(_stack) > _base_depth[0]:
                rel(_stack[-1])
            raise _Stop

        pers = apool(name="pers", bufs=1)
        small = apool(name="small", bufs=1)
        scr = apool(name="scr", bufs=2)
        psb = apool(name="psb", bufs=3, space="PSUM")
        p_kv = apool(name="p_kv", bufs=1)

        # ---------- persistent loads ----------
        x_sb = pers.tile([128, CT, L], f32)
        h = pers.tile([128, CT, L], f32r)

        mask_sb = small.tile([128, CT, G], f32)
        dma(out=mask_sb, in_=mask_d)
        maskT_sb = small.tile([G, C], f32)
        dma(out=maskT_sb, in_=maskT_d)
        gn1g = small.tile([128, CT], f32); dma(out=gn1g, in_=gn1g_d)
        gn1b = small.tile([128, CT], f32); dma(out=gn1b, in_=gn1b_d)
        gn2g = small.tile([128, CT], f32); dma(out=gn2g, in_=gn2g_d)
        gn2b = small.tile([128, CT], f32); dma(out=gn2b, in_=gn2b_d)
        qkb = small.tile([128, 2 * CT], f32); dma(out=qkb, in_=qkb_d)
        vb_row = small.tile([1, C], f32); dma(out=vb_row, in_=vb_row_d)
        sapb = small.tile([128, CT], f32); dma(out=sapb, in_=sapb_d)
        qb = small.tile([128, CT], f32); dma(out=qb, in_=qb_d)
        kb = small.tile([128, CT], f32); dma(out=kb, in_=kb_d)
        vb2_row = small.tile([1, C], f32); dma(out=vb2_row, in_=vb2_row_d)
        capb = small.tile([128, CT], f32); dma(out=capb, in_=capb_d)
        b1 = small.tile([128, FT], f32); dma(out=b1, in_=b1_d)
        b2 = small.tile([128, CT], f32); dma(out=b2, in_=b2_d)
        smask = small.tile([128, 1], f32); dma(out=smask, in_=smask_d)

        vb_bc = small.tile([128, C], f32)
        nc.gpsimd.partition_broadcast(vb_bc, vb_row)
        vb2_bc = small.tile([128, C], f32)
        nc.gpsimd.partition_broadcast(vb2_bc, vb2_row)

        eps_t = small.tile([G, 1], f32)
        nc.vector.memset(eps_t, EPS)
        ones_t = small.tile([128, 1], f32)
        nc.vector.memset(ones_t, 1.0)
        zeros_t = small.tile([128, 1], f32)
        nc.vector.memset(zeros_t, 0.0)

        # cross-attention K/V live here across the whole pass
        k2 = p_kv.tile([128, CT, SP], bf16)
        v2_aug = p_kv.tile([128, NH * (HD + 1)], bf16)

        # ---------- phase 0: cross-attn K/V from context (before x arrives) ----------
        p_ctxw = apool(name="p_ctxw", bufs=1)
        ctxT = p_ctxw.tile([128, KTC, SP], bf16)
        dma(out=ctxT, in_=ctxT_d)
        kwT = p_ctxw.tile([128, KTC, C], bf16)
        dma(out=kwT, in_=kwT_d)
        vwT = p_ctxw.tile([128, KTC, C], bf16)
        dma(out=vwT, in_=vwT_d)

        for ct in range(CT):
            ps = psb.tile([128, SP], f32, tag="av", bufs=2, name=f"k2ps{ct}")
            for kt in range(KTC):
                nc.tensor.matmul(ps, kwT[:, kt, ct * 128:(ct + 1) * 128],
                                 ctxT[:, kt, :], start=(kt == 0), stop=(kt == KTC - 1))
            nc.vector.tensor_scalar_add(out=k2[:, ct, :], in0=ps, scalar1=kb[:, ct:ct + 1])
        nc.vector.tensor_copy(out=k2[:, :, S:SP],
                              in_=zeros_t.to_broadcast([128, CT, SP - S]))

        ps_v2 = psb.tile([128, C], f32, tag="ps", bufs=3)
        for kt in range(KTC):
            nc.tensor.matmul(ps_v2, ctxT[:, kt, :], vwT[:, kt, :],
                             start=(kt == 0), stop=(kt == KTC - 1))
        v2t = scr.tile([128, C], f32, tag="v2t")
        nc.vector.tensor_add(v2t, ps_v2, vb2_bc)
        nc.vector.tensor_scalar_mul(
            out=v2_aug.rearrange("p (h e) -> p h e", e=HD + 1)[:, :, 0:HD],
            in0=v2t.rearrange("p (h e) -> p h e", e=HD), scalar1=smask)
        nc.vector.tensor_copy(
            out=v2_aug.rearrange("p (h e) -> p h e", e=HD + 1)[:, :, HD:HD + 1],
            in_=smask.to_broadcast([128, NH, 1]))
        rel(p_ctxw)

        for ct in range(CT):
            dma(out=x_sb[:, ct, :], in_=x_d[:, ct, :])

        # ---------- GroupNorm helper ----------
        def groupnorm(src, dst, g_sb, b_sb, src_f32=False):
            cast = (lambda ap: ap) if src_f32 else (lambda ap: ap.bitcast(f32))
            stats = small.tile([128, CT, 2], f32, tag="gn_stats")
            for ct in range(CT):
                nc.vector.reduce_sum(out=stats[:, ct, 0:1], in_=cast(src[:, ct, :]), axis=AX)
            for ct in range(CT):
                sc = scr.tile([128, L], f32, tag="gn_scr", bufs=1)
                nc.scalar.activation(out=sc, in_=cast(src[:, ct, :]), func=Square,
                                     accum_out=stats[:, ct, 1:2])
            psg = psb.tile([G, 2], f32, tag="av", bufs=2)
            for ct in range(CT):
                nc.tensor.matmul(psg, mask_sb[:, ct, :], stats[:, ct, :],
                                 start=(ct == 0), stop=(ct == CT - 1))
            mv = small.tile([G, 2], f32, tag="gn_mv")
            nc.vector.tensor_scalar_mul(mv, psg, 1.0 / (16 * L))
            tmp = small.tile([G, 1], f32, tag="gn_tmp")
            nc.vector.tensor_mul(tmp, mv[:, 0:1], mv[:, 0:1])
            nc.vector.tensor_sub(mv[:, 1:2], mv[:, 1:2], tmp)
            sq = small.tile([G, 1], f32, tag="gn_sq")
            nc.scalar.activation(out=sq, in_=mv[:, 1:2], func=Sqrt, bias=eps_t)
            nc.vector.reciprocal(mv[:, 1:2], sq)
            ss = small.tile([128, CT, 2], f32, tag="gn_ss")
            for ct in range(CT):
                pc = psb.tile([128, 2], f32, tag="av", bufs=2)
                nc.tensor.matmul(pc, maskT_sb[:, ct * 128:(ct + 1) * 128], mv,
                                 start=True, stop=True)
                nc.vector.tensor_mul(ss[:, ct, 0:1], pc[:, 1:2], g_sb[:, ct:ct + 1])
                t2 = small.tile([128, 1], f32, tag="gn_t2")
                nc.vector.tensor_mul(t2, pc[:, 0:1], ss[:, ct, 0:1])
                nc.vector.tensor_sub(ss[:, ct, 1:2], b_sb[:, ct:ct + 1], t2)
            for ct in range(CT):
                nc.vector.tensor_scalar(
                    out=dst[:, ct, :], in0=cast(src[:, ct, :]),
                    scalar1=ss[:, ct, 0:1], scalar2=ss[:, ct, 1:2],
                    op0=mult, op1=add)

        _base_depth[0] = len(_stack)
        for _rep in range(repeat):
          try:
            # ---------- phase 1: the two GroupNorms ----------
            groupnorm(x_sb, h, gn1g, gn1b, src_f32=True)
            if stop_after == "gn1":
                stop_dump(h)

            p_ao = apool(name="p_ao", bufs=1)
            attn_out = p_ao.tile([128, CT, L], bf16)
            p_sap = apool(name="p_sap", bufs=1)
            sapT = p_sap.tile([128, CT, C], bf16)
            dma(out=sapT, in_=sapT_d)
            p_vaug = apool(name="p_vaug", bufs=1)
            v_aug = p_vaug.tile([128, LT, NH * (HD + 1)], bf16)
            p_qk = apool(name="p_qk", bufs=1)
            qk = p_qk.tile([128, 2 * CT, L], bf16)      # q tiles 0-3, k tiles 4-7

            if gn2_skip:
                hn = h          # sa_gn is identity and gn_in output is normalized
            else:
                p_hn = apool(name="p_hn", bufs=1)
                hn = p_hn.tile([128, CT, L], f32r)
                groupnorm(h, hn, gn2g, gn2b)

            # ---------- phase 2a: qkv ----------
            p_wqkv = apool(name="p_wqkv", bufs=1)
            qkvwT = p_wqkv.tile([128, CT, 3 * C], f32r)
            dma(out=qkvwT, in_=qkvwT_d)

            p_pt = apool(name="p_pt", bufs=3)

            def sa_scores(hp):
                """S^T then exp for head pair (2hp, 2hp+1), row-group packed."""
                pts = [p_pt.tile([128, LT, L], bf16, tag="PT", bufs=3,
                                 name=f"pt{hp}_{i}") for i in range(2)]
                kt_ = 4 + hp
                for mt in range(LT):
                    pp = [psb.tile([128, L], f32, tag="ps", bufs=3,
                                   name=f"sps{hp}_{mt}_{i}") for i in range(2)]
                    for ch in range(NCH):
                        for i, po in ((0, 0), (1, 64)):
                            nc.tensor.matmul(
                                pp[i][:, ch * CH:(ch + 1) * CH],
                                qk[po:po + 64, kt_, mt * 128:(mt + 1) * 128],
                                qk[po:po + 64, hp, ch * CH:(ch + 1) * CH],
                                start=True, stop=True)
                    for i in range(2):
                        nc.scalar.activation(out=pts[i][:, mt, :], in_=pp[i],
                                             func=Exp, scale=SCALE)
                return pts

            def qkv_group(mt):
                ps = psb.tile([128, L], f32, tag="ps", bufs=3, name=f"qkps{mt}")
                for kt in range(CT):
                    for ch in range(NCH):
                        nc.tensor.matmul(ps[:, ch * CH:(ch + 1) * CH],
                                         qkvwT[:, kt, mt * 128:(mt + 1) * 128],
                                         hn[:, kt, ch * CH:(ch + 1) * CH],
                                         start=(kt == 0), stop=(kt == CT - 1))
                nc.vector.tensor_scalar_add(out=qk[:, mt, :], in0=ps,
                                            scalar1=qkb[:, mt:mt + 1])

            for hp in range(CT):                        # q/k paired per head pair
                qkv_group(hp)
                qkv_group(4 + hp)
            # v in transposed (sequence) layout, augmented with a ones column
            for mt in range(LT):
                ps = psb.tile([128, C], f32, tag="ps", bufs=3, name=f"vps{mt}")
                for kt in range(CT):
                    nc.tensor.matmul(ps, hn[:, kt, mt * 128:(mt + 1) * 128],
                                     qkvwT[:, kt, 2 * C:3 * C],
                                     start=(kt == 0), stop=(kt == CT - 1))
                nc.vector.tensor_add(
                    out=v_aug[:, mt, :].rearrange("p (h e) -> p h e", e=HD + 1)[:, :, 0:HD],
                    in0=ps.rearrange("p (h e) -> p h e", e=HD),
                    in1=vb_bc.rearrange("p (h e) -> p h e", e=HD))
            nc.vector.tensor_copy(
                out=v_aug.rearrange("p m (h e) -> p m h e", e=HD + 1)[:, :, :, HD:HD + 1],
                in_=ones_t.to_broadcast([128, LT, NH, 1]))
            pts0 = sa_scores(0)

            if stop_after == "qkv":
                stop_dump(qk[:, 0:CT, :])

            # ---------- phase 2b: self-attention ----------
            def sa_av(hp, pts):
                for i in range(2):
                    hh = 2 * hp + i
                    for ch in range(NCH):
                        ps = psb.tile([HD + 1, CH], f32, tag="av", bufs=2,
                                      name=f"avps{hh}_{ch}")
                        for mt in range(LT):
                            nc.tensor.matmul(
                                ps, v_aug[:, mt, hh * (HD + 1):(hh + 1) * (HD + 1)],
                                pts[i][:, mt, ch * CH:(ch + 1) * CH],
                                start=(mt == 0), stop=(mt == LT - 1))
                        rec = scr.tile([1, CH], f32, tag="rec", bufs=6)
                        nc.vector.reciprocal(rec, ps[HD:HD + 1, :])
                        rb = scr.tile([HD, CH], f32, tag="recb", bufs=6)
                        nc.gpsimd.partition_broadcast(rb, rec)
                        nc.vector.tensor_mul(
                            out=attn_out[64 * i:64 * i + 64, hp, ch * CH:(ch + 1) * CH],
                            in0=ps[0:HD, :], in1=rb)

            prev = (0, pts0)
            for hp in range(1, CT):
                pts = sa_scores(hp)
                sa_av(*prev)
                prev = (hp, pts)
            sa_av(*prev)
            rel(p_pt)
            rel(p_wqkv)
            if not gn2_skip:
                rel(p_hn)
            rel(p_qk)
            rel(p_vaug)

            # sa_proj + residual (h += proj(attn_out) + b)
            for ct in range(CT):
                for ch in range(NCH):
                    ps = psb.tile([128, CH], f32, tag="ps", bufs=3,
                                  name=f"sap{ct}_{ch}")
                    for kt in range(CT):
                        nc.tensor.matmul(ps, sapT[:, kt, ct * 128:(ct + 1) * 128],
                                         attn_out[:, kt, ch * CH:(ch + 1) * CH],
                                         start=(kt == 0), stop=(kt == CT - 1))
                    nc.vector.scalar_tensor_tensor(
                        out=h[:, ct, ch * CH:(ch + 1) * CH], in0=ps,
                        scalar=sapb[:, ct:ct + 1],
                        in1=h[:, ct, ch * CH:(ch + 1) * CH].bitcast(f32),
                        op0=add, op1=add)
            rel(p_sap)
            rel(p_ao)
            if stop_after == "sa":
                stop_dump(h)

            # ---------- phase 3: cross-attention ----------
            p_caa = apool(name="p_caa", bufs=1)
            q2 = p_caa.tile([128, CT, L], bf16)
            ca_out = p_caa.tile([128, CT, L], bf16)
            p_w1 = apool(name="p_w1", bufs=1)
            w1T = p_w1.tile([128, CT, FF], f32r)
            p_qcw = apool(name="p_qcw", bufs=1)
            qwT = p_qcw.tile([128, CT, C], f32r); dma(out=qwT, in_=qwT_d)
            capT = p_qcw.tile([128, CT, C], bf16); dma(out=capT, in_=capT_d)
            dma(out=w1T, in_=w1T_d)                     # prefetch during CA
            p_p2 = apool(name="p_p2", bufs=4)

            # q2 = q_w @ h (interleaved with scores below)
            def q2_group(ct):
                ps = psb.tile([128, L], f32, tag="ps", bufs=3, name=f"q2ps{ct}")
                for kt in range(CT):
                    for ch in range(NCH):
                        nc.tensor.matmul(ps[:, ch * CH:(ch + 1) * CH],
                                         qwT[:, kt, ct * 128:(ct + 1) * 128],
                                         h[:, kt, ch * CH:(ch + 1) * CH],
                                         start=(kt == 0), stop=(kt == CT - 1))
                nc.vector.tensor_scalar_add(out=q2[:, ct, :], in0=ps,
                                            scalar1=qb[:, ct:ct + 1])

            def ca_scores(hp):
                pp = [psb.tile([128, L], f32, tag="ps", bufs=3,
                               name=f"cps{hp}_{i}") for i in range(2)]
                for ch in range(NCH):
                    for i, po in ((0, 0), (1, 64)):
                        nc.tensor.matmul(pp[i][:, ch * CH:(ch + 1) * CH],
                                         k2[po:po + 64, hp, :],
                                         q2[po:po + 64, hp, ch * CH:(ch + 1) * CH],
                                         start=True, stop=True)
                p2s = []
                for i in range(2):
                    p2 = p_p2.tile([128, L], bf16, tag="P2", bufs=8, name=f"p2_{hp}_{i}")
                    nc.scalar.activation(out=p2, in_=pp[i], func=Exp, scale=SCALE)
                    p2s.append(p2)
                return p2s

            def ca_av(hp, p2s):
                for i in range(2):
                    hh = 2 * hp + i
                    for ch in range(NCH):
                        ps2 = psb.tile([HD + 1, CH], f32, tag="av", bufs=2,
                                       name=f"avp2_{hh}_{ch}")
                        nc.tensor.matmul(ps2, v2_aug[:, hh * (HD + 1):(hh + 1) * (HD + 1)],
                                         p2s[i][:, ch * CH:(ch + 1) * CH],
                                         start=True, stop=True)
                        rec = scr.tile([1, CH], f32, tag="rec", bufs=6)
                        nc.vector.reciprocal(rec, ps2[HD:HD + 1, :])
                        rb = scr.tile([HD, CH], f32, tag="recb", bufs=6)
                        nc.gpsimd.partition_broadcast(rb, rec)
                        nc.vector.tensor_mul(
                            out=ca_out[64 * i:64 * i + 64, hp, ch * CH:(ch + 1) * CH],
                            in0=ps2[0:HD, :], in1=rb)

            all_p2 = []
            for hp in range(CT):
                q2_group(hp)
                all_p2.append(ca_scores(hp))
            for hp in range(CT):
                ca_av(hp, all_p2[hp])

            # ca_proj + residual
            for ct in range(CT):
                for ch in range(NCH):
                    ps = psb.tile([128, CH], f32, tag="ps", bufs=3,
                                  name=f"cap{ct}_{ch}")
                    for kt in range(CT):
                        nc.tensor.matmul(ps, capT[:, kt, ct * 128:(ct + 1) * 128],
                                         ca_out[:, kt, ch * CH:(ch + 1) * CH],
                                         start=(kt == 0), stop=(kt == CT - 1))
                    nc.vector.scalar_tensor_tensor(
                        out=h[:, ct, ch * CH:(ch + 1) * CH], in0=ps,
                        scalar=capb[:, ct:ct + 1],
                        in1=h[:, ct, ch * CH:(ch + 1) * CH].bitcast(f32),
                        op0=add, op1=add)
            rel(p_p2)
            rel(p_qcw)
            if stop_after == "ca":
                stop_dump(h)
            for ct in range(CT):
                nc.vector.tensor_add(x_sb[:, ct, :], h[:, ct, :].bitcast(f32),
                                     x_sb[:, ct, :])

            # ---------- phase 4: FFN ----------
            p_w2 = apool(name="p_w2", bufs=1)
            w2T = p_w2.tile([128, FT, C], bf16)
            dma(out=w2T, in_=w2T_d)
            p_ff = apool(name="p_ff", bufs=1)
            ff1 = p_ff.tile([128, FT, L], bf16)
            p_of = apool(name="p_of", bufs=2)

            for ft in range(FT):
                ps = psb.tile([128, L], f32, tag="ps", bufs=3, name=f"f1ps{ft}")
                for kt in range(CT):
                    for ch in range(NCH):
                        nc.tensor.matmul(ps[:, ch * CH:(ch + 1) * CH],
                                         w1T[:, kt, ft * 128:(ft + 1) * 128],
                                         h[:, kt, ch * CH:(ch + 1) * CH],
                                         start=(kt == 0), stop=(kt == CT - 1))
                nc.scalar.activation(out=ff1[:, ft, :], in_=ps, func=Gelu,
                                     bias=b1[:, ft:ft + 1], scale=1.0)
            for ct in range(CT):
                for ch in range(NCH):
                    ps = psb.tile([128, CH], f32, tag="av", bufs=2,
                                  name=f"f2ps{ct}_{ch}")
                    for kt in range(FT):
                        nc.tensor.matmul(ps, w2T[:, kt, ct * 128:(ct + 1) * 128],
                                         ff1[:, kt, ch * CH:(ch + 1) * CH],
                                         start=(kt == 0), stop=(kt == FT - 1))
                    of = p_of.tile([128, CH], f32, tag="of")
                    nc.vector.scalar_tensor_tensor(
                        out=of, in0=ps, scalar=b2[:, ct:ct + 1],
                        in1=x_sb[:, ct, ch * CH:(ch + 1) * CH],
                        op0=add, op1=add)
                    dma(out=out_d[:, ct, ch * CH:(ch + 1) * CH], in_=of)

            for p in (p_of, p_ff, p_w2, p_w1, p_caa):
                rel(p)
          except _Stop:
            pass
        for p in (p_kv, psb, scr, small, pers):
            rel(p)

    nc.compile()
    return nc


def _tileK(wT, kt, dt=np.float32):
    """[K, F] -> [128, kt, F] partition-major layout."""
    K, F = wT.shape
    return np.ascontiguousarray(
        wT.reshape(kt, 128, F).transpose(1, 0, 2)).astype(dt)


def _conv(b):
    """[n] -> [128, n//128] conv-layout bias."""
    return np.ascontiguousarray(np.asarray(b, np.float32).reshape(-1, 128).T)


def prepare_in_maps(inputs):
    import ml_dtypes
    bf = ml_dtypes.bfloat16
    f = lambda a: np.asarray(a, np.float32)
    x = f(inputs["x"]); ctx = f(inputs["context"])
    shared = {
        "qkv_wT": _tileK(f(inputs["qkv_w"]).T, CT),
        "sa_proj_wT": _tileK(f(inputs["sa_proj_w"]).T, CT, bf),
        "q_wT": _tileK(f(inputs["q_w"]).T, CT),
        "k_wT": _tileK(f(inputs["k_w"]).T, KTC, bf),
        "v_wT": _tileK(f(inputs["v_w"]).T, KTC, bf),
        "ca_proj_wT": _tileK(f(inputs["ca_proj_w"]).T, CT, bf),
        "w1T": _tileK(f(inputs["w1"]).T, CT),
        "w2T": _tileK(f(inputs["w2"]).T, FT, bf),
        "gn1g": _conv(inputs["gn_in_g"]), "gn1b": _conv(inputs["gn_in_b"]),
        "gn2g": _conv(inputs["sa_gn_g"]), "gn2b": _conv(inputs["sa_gn_b"]),
        "qkb": _conv(f(inputs["qkv_b"])[:2 * C]),
        "vb_row": f(inputs["qkv_b"])[2 * C:].reshape(1, C).copy(),
        "sapb": _conv(inputs["sa_proj_b"]),
        "qb": _conv(inputs["q_b"]), "kb": _conv(inputs["k_b"]),
        "vb2_row": f(inputs["v_b"]).reshape(1, C).copy(),
        "capb": _conv(inputs["ca_proj_b"]),
        "b1": _conv(inputs["b1"]), "b2": _conv(inputs["b2"]),
    }
    cidx = np.arange(C) // 16
    mask = (cidx[:, None] == np.arange(G)[None, :]).astype(np.float32)  # [C, G]
    shared["gn_mask"] = np.ascontiguousarray(
        mask.reshape(CT, 128, G).transpose(1, 0, 2))
    shared["gn_maskT"] = np.ascontiguousarray(mask.T)
    shared["smask"] = (np.arange(SP) < S).astype(np.float32).reshape(SP, 1)

    in_maps = []
    for b in range(B):
        xb = np.ascontiguousarray(
            x[b].reshape(C, L).reshape(CT, 128, L).transpose(1, 0, 2))
        ctxT = np.zeros((CTX, SP), np.float32)
        ctxT[:, :S] = ctx[b].T
        ctxTb = np.ascontiguousarray(
            ctxT.reshape(KTC, 128, SP).transpose(1, 0, 2)).astype(bf)
        in_maps.append({"x": xb, "ctxT": ctxTb, **shared})
    return in_maps


def kernel(**inputs):
    from concourse.bass_utils import run_bass_kernel_spmd
    if "nc" not in _CACHE:
        _CACHE["nc"] = _build()
    nc = _CACHE["nc"]
    in_maps = prepare_in_maps(inputs)
    res = run_bass_kernel_spmd(nc, in_maps, core_ids=list(range(B)))
    out = np.stack([
        np.ascontiguousarray(res.results[b]["out"].transpose(1, 0, 2)).reshape(C, H, W)
        for b in range(B)])
    return out.astype(np.float32)



# revision 99
# speedup vs baseline: 1.1981x; 1.1981x over previous
"""Trainium2 Bass kernel for nn_AttentionBlock (GN + self-attn + cross-attn + FFN).

Sharding: data-parallel over batch B=8 -> one batch element per NeuronCore.
Per-core layout: activations as [C(partitions), L(free)] "conv" layout.
Attention computed with transposed scores S^T[m, l]; softmax sums come from an
augmented-V matmul (extra ones column -> Z lands in psum partition 64), so no
cross-partition reductions or transposes are needed. Row-softmax max-subtraction
is skipped (logits are provably < 2 for this block's scale).

vs baseline: the two GroupNorms share a single stats pass (GN2's group stats
are derived algebraically from GN1's per-channel sums), weights load in bf16,
params are packed into one DMA blob, and CA/FFN are pipelined by l-chunk so
the tensor engine is not starved behind the softmax-exp wall.
"""
import sys

for _p in ("/opt/trn_rl_repo", "/root/.axon_site/_ro/trn_rl_repo"):
    if _p not in sys.path:
        sys.path.append(_p)

import numpy as np

# ---- problem constants (hardcoded per contract) ----
B, C, H, W = 8, 512, 32, 32
L = H * W                       # 1024
NH, HD = 8, 64
CT = C // 128                   # 4 channel tiles
LT = L // 128                   # 8 l/m tiles
NCH = 2                         # l chunks of 512
CH = L // NCH                   # 512
CTX = 768
S = 77
SP = 128                        # padded context tokens
KTC = CTX // 128                # 6
FF = 4 * C                      # 2048
FT = FF // 128                  # 16
G = 32                          # groups
EPS = 1e-5
SCALE = HD ** -0.5

# blob layout (f32 columns in the packed parameter tensor)
_BL = {}
_off = 0
for _name, _w in (("gn1g", CT), ("gn1b", CT), ("gn2g", CT), ("gn2b", CT),
                  ("qkb", 2 * CT), ("sapb", CT), ("qb", CT), ("kb", CT),
                  ("capb", CT), ("b2", CT), ("b1", FT), ("smask", 1),
                  ("mask", CT * G)):
    _BL[_name] = (_off, _off + _w)
    _off += _w
NBLOB = _off

_CACHE = {}


def _build(gelu_identity=False, stop_after=None, repeat=1, gn2_skip=False):
    import concourse.mybir as mybir
    import concourse.tile as tile
    from concourse import bacc

    f32 = mybir.dt.float32
    f32r = mybir.dt.float32r
    bf16 = mybir.dt.bfloat16
    fp8 = mybir.dt.float8e4
    Exp = mybir.ActivationFunctionType.Exp
    Gelu = (mybir.ActivationFunctionType.Identity if gelu_identity
            else mybir.ActivationFunctionType.Gelu)
    Ln = mybir.ActivationFunctionType.Ln
    Square = mybir.ActivationFunctionType.Square
    Copy = mybir.ActivationFunctionType.Copy
    DoubleRow = mybir.MatmulPerfMode.DoubleRow
    add = mybir.AluOpType.add
    mult = mybir.AluOpType.mult
    AX = mybir.AxisListType.X

    nc = bacc.Bacc("TRN2", target_bir_lowering=False, debug=False, num_devices=8)

    def din(name, shape, dt=bf16):
        return nc.dram_tensor(name, shape, dt, kind="ExternalInput").ap()

    x_d = din("x", [128, CT, L], f32)
    ctxT_d = din("ctxT", [128, KTC, SP], bf16)
    blob_d = din("blob", [128, NBLOB], f32)
    maskT_d = din("gn_maskT", [G, C], f32)
    vbrows_d = din("vbrows", [1, 2 * C], f32)
    qkvwT_d = din("qkv_wT", [128, CT, 3 * C], fp8)   # scaled x8 on host
    sapT_d = din("sa_proj_wT", [128, CT, C], fp8)
    qwT_d = din("q_wT", [128, CT, C], fp8)   # scaled x8 on host
    kwT_d = din("k_wT", [128, KTC, C])
    vwT_d = din("v_wT", [128, KTC, C])
    capT_d = din("ca_proj_wT", [128, CT, C], fp8)
    w1T_d = din("w1T", [128, CT, FF], fp8)   # scaled x8 on host
    w2T_d = din("w2T", [128, FT, C], fp8)

    out_d = nc.dram_tensor("out", [128, CT, L], f32, kind="ExternalOutput").ap()

    dma = nc.sync.dma_start

    class _Stop(Exception):
        pass

    with tile.TileContext(nc) as tc:
        _stack = []

        def apool(**kw):
            p = tc.alloc_tile_pool(**kw)
            _stack.append(p)
            return p

        def rel(p):
            assert _stack[-1] is p
            _stack.pop()
            p.release()

        _base_depth = [0]

        def stop_dump(src):
            """Truncated build: dump src, unwind pools opened within this pass."""
            for ct in range(CT):
                dma(out=out_d[:, ct, :], in_=src[:, ct, :].bitcast(f32))
            while len(_stack) > _base_depth[0]:
                rel(_stack[-1])
            raise _Stop

        pers = apool(name="pers", bufs=1)
        small = apool(name="small", bufs=1)
        scr = apool(name="scr", bufs=2)
        psb = apool(name="psb", bufs=3, space="PSUM")
        p_kv = apool(name="p_kv", bufs=1)

        # ---------- persistent loads (program order == DMA priority) ----------
        x_sb = pers.tile([128, CT, L], f32)
        h = pers.tile([128, CT, L], f32r)
        for ct in range(CT):
            dma(out=x_sb[:, ct, :], in_=x_d[:, ct, :])

        blob = small.tile([128, NBLOB], f32)
        dma(out=blob, in_=blob_d)
        maskT_sb = small.tile([G, C], f32)
        dma(out=maskT_sb, in_=maskT_d)
        vbrows = small.tile([1, 2 * C], f32)
        dma(out=vbrows, in_=vbrows_d)

        p_ctxw = apool(name="p_ctxw", bufs=1)
        ctxT = p_ctxw.tile([128, KTC, SP], bf16)
        dma(out=ctxT, in_=ctxT_d)
        kwT = p_ctxw.tile([128, KTC, C], bf16)
        dma(out=kwT, in_=kwT_d)
        vwT = p_ctxw.tile([128, KTC, C], bf16)
        dma(out=vwT, in_=vwT_d)

        def bl(name):
            a, b = _BL[name]
            return blob[:, a:b]

        gn1g, gn1b = bl("gn1g"), bl("gn1b")
        gn2g, gn2b = bl("gn2g"), bl("gn2b")
        qkb, sapb, qb, kb = bl("qkb"), bl("sapb"), bl("qb"), bl("kb")
        capb, b2, b1, smask = bl("capb"), bl("b2"), bl("b1"), bl("smask")
        mask_sb = bl("mask").rearrange("p (c g) -> p c g", g=G)

        vb_bc = small.tile([128, C], f32)
        nc.gpsimd.partition_broadcast(vb_bc, vbrows[0:1, 0:C])
        vb2_bc = small.tile([128, C], f32)
        nc.gpsimd.partition_broadcast(vb2_bc, vbrows[0:1, C:2 * C])

        eps_t = small.tile([G, 1], f32)
        nc.vector.memset(eps_t, EPS)
        ones_t = small.tile([128, 1], f32)
        nc.vector.memset(ones_t, 1.0)
        zeros_t = small.tile([128, 1], f32)
        nc.vector.memset(zeros_t, 0.0)

        # cross-attention K/V live here across the whole pass
        k2 = p_kv.tile([128, CT, SP], bf16)
        v2_aug = p_kv.tile([128, NH * (HD + 1)], fp8)

        # ---------- phase 0: cross-attn K/V from context ----------
        # Emitted inside rep 0 AFTER the GN stats matmuls: the PE queue is
        # strict in-order, and k2/v2 wait on late-arriving context weights —
        # putting them first would head-of-line-block the GN matmuls.
        Ident = mybir.ActivationFunctionType.Identity

        def phase0():
            for ct in range(CT):
                ps = psb.tile([128, SP], f32, tag="av", bufs=2, name=f"k2ps{ct}")
                for kt in range(KTC):
                    nc.tensor.matmul(ps, kwT[:, kt, ct * 128:(ct + 1) * 128],
                                     ctxT[:, kt, :], start=(kt == 0),
                                     stop=(kt == KTC - 1))
                nc.scalar.activation(out=k2[:, ct, :], in_=ps, func=Ident,
                                     bias=kb[:, ct:ct + 1], scale=1.0)
            nc.gpsimd.tensor_copy(out=k2[:, :, S:SP],
                                  in_=zeros_t.to_broadcast([128, CT, SP - S]))

            ps_v2 = psb.tile([128, C], f32, tag="ps", bufs=3)
            for kt in range(KTC):
                nc.tensor.matmul(ps_v2, ctxT[:, kt, :], vwT[:, kt, :],
                                 start=(kt == 0), stop=(kt == KTC - 1))
            v2t = scr.tile([128, C], f32, tag="v2t")
            nc.vector.tensor_add(v2t, ps_v2, vb2_bc)
            nc.gpsimd.tensor_scalar_mul(
                out=v2_aug.rearrange("p (h e) -> p h e", e=HD + 1)[:, :, 0:HD],
                in0=v2t.rearrange("p (h e) -> p h e", e=HD), scalar1=smask)
            nc.gpsimd.tensor_copy(
                out=v2_aug.rearrange("p (h e) -> p h e", e=HD + 1)[:, :, HD:HD + 1],
                in_=smask.to_broadcast([128, NH, 1]))

        # ---------- fused GroupNorm (GN1 stats -> GN2 stats analytically) ----
        gn_stats_t = small.tile([128, CT, 2], f32)

        def gn_stats(ct, cold=False):
            """Per-channel sum/sumsq of x_sb tile ct. Cold path (rep 0 prefix,
            ACT idle) uses the ACT Square accumulator; the software-pipelined
            path runs during the previous rep's FFN where ACT is saturated, so
            x^2 goes to Pool with the reduce on DVE."""
            nc.vector.reduce_sum(out=gn_stats_t[:, ct, 0:1], in_=x_sb[:, ct, :],
                                 axis=AX)
            sc = scr.tile([128, L], bf16, tag="gn_scr", bufs=2)
            nc.scalar.activation(out=sc, in_=x_sb[:, ct, :], func=Square,
                                 accum_out=gn_stats_t[:, ct, 1:2])

        def gn_fused(dst_h, dst_hn):
            """dst_h = GN1(x_sb); dst_hn = GN2(GN1(x_sb)) in ONE data pass for
            stats: GN2 group stats derive from GN1's per-channel S/SS."""
            stats = gn_stats_t
            psg = psb.tile([G, 2], f32, tag="av", bufs=2)
            for ct in range(CT):
                nc.tensor.matmul(psg, mask_sb[:, ct, :], stats[:, ct, :],
                                 start=(ct == 0), stop=(ct == CT - 1))
            mv = small.tile([G, 2], f32, tag="gn_mv")
            nc.vector.tensor_scalar_mul(mv, psg, 1.0 / (16 * L))
            tmpg = small.tile([G, 1], f32, tag="gn_tmpg")
            nc.vector.tensor_mul(tmpg, mv[:, 0:1], mv[:, 0:1])
            nc.vector.tensor_sub(mv[:, 1:2], mv[:, 1:2], tmpg)
            # inv-std = exp(-0.5*ln(V+eps)): stays in the exp/ln table set so
            # no sqrt-table load interrupts the ACT pipeline
            sqg = small.tile([G, 1], f32, tag="gn_sqg")
            nc.scalar.activation(out=sqg, in_=mv[:, 1:2], func=Ln, bias=eps_t)
            nc.scalar.activation(out=mv[:, 1:2], in_=sqg, func=Exp, scale=-0.5)
            # per-channel (M1, inv1)
            pc = psb.tile([128, CT, 2], f32, tag="av", bufs=2)
            for ct in range(CT):
                nc.tensor.matmul(pc[:, ct, :], maskT_sb[:, ct * 128:(ct + 1) * 128],
                                 mv, start=True, stop=True)
            ss = small.tile([128, CT, 2], f32, tag="gn_ss")      # a, d
            a_, d_ = ss[:, :, 0:1], ss[:, :, 1:2]
            g1v = gn1g.rearrange("p (c o) -> p c o", o=1)
            b1v = gn1b.rearrange("p (c o) -> p c o", o=1)
            t1 = small.tile([128, CT, 1], f32, tag="gn_t1")
            t2 = small.tile([128, CT, 1], f32, tag="gn_t2")
            t3 = small.tile([128, CT, 1], f32, tag="gn_t3")
            nc.vector.tensor_mul(a_, pc[:, :, 1:2], g1v)          # a = inv1*g1
            nc.vector.tensor_mul(t1, pc[:, :, 0:1], a_)
            nc.vector.tensor_sub(d_, b1v, t1)                     # d = b1 - M1*a
            if gn2_skip:
                for ct in range(CT):
                    nc.vector.tensor_scalar(
                        out=dst_h[:, ct, :], in0=x_sb[:, ct, :],
                        scalar1=ss[:, ct, 0:1], scalar2=ss[:, ct, 1:2],
                        op0=mult, op1=add)
                return
            # per-channel stats of h: Sh = a*S + L*d ; SSh = a*(a*SS + 2dS) + L*d^2
            st2 = small.tile([128, CT, 2], f32, tag="gn_st2")
            nc.vector.tensor_mul(t1, a_, stats[:, :, 0:1])        # aS
            nc.vector.tensor_scalar_mul(out=t2, in0=d_, scalar1=float(L))
            nc.vector.tensor_add(st2[:, :, 0:1], t1, t2)          # Sh
            nc.vector.tensor_mul(t1, a_, stats[:, :, 1:2])        # a*SS
            nc.vector.tensor_mul(t2, d_, stats[:, :, 0:1])        # dS
            nc.vector.tensor_scalar_mul(out=t2, in0=t2, scalar1=2.0)
            nc.vector.tensor_add(t1, t1, t2)
            nc.vector.tensor_mul(t1, a_, t1)                      # a*(aSS+2dS)
            nc.vector.tensor_mul(t3, d_, d_)
            nc.vector.tensor_scalar_mul(out=t3, in0=t3, scalar1=float(L))
            nc.vector.tensor_add(st2[:, :, 1:2], t1, t3)          # SSh
            psg2 = psb.tile([G, 2], f32, tag="av", bufs=2)
            for ct in range(CT):
                nc.tensor.matmul(psg2, mask_sb[:, ct, :], st2[:, ct, :],
                                 start=(ct == 0), stop=(ct == CT - 1))
            mv2 = small.tile([G, 2], f32, tag="gn_mv2")
            nc.vector.tensor_scalar_mul(mv2, psg2, 1.0 / (16 * L))
            nc.vector.tensor_mul(tmpg, mv2[:, 0:1], mv2[:, 0:1])
            nc.vector.tensor_sub(mv2[:, 1:2], mv2[:, 1:2], tmpg)
            nc.scalar.activation(out=sqg, in_=mv2[:, 1:2], func=Ln, bias=eps_t)
            nc.scalar.activation(out=mv2[:, 1:2], in_=sqg, func=Exp, scale=-0.5)
            pc2 = psb.tile([128, CT, 2], f32, tag="av", bufs=2)
            for ct in range(CT):
                nc.tensor.matmul(pc2[:, ct, :], maskT_sb[:, ct * 128:(ct + 1) * 128],
                                 mv2, start=True, stop=True)
            AB = small.tile([128, CT, 2], f32, tag="gn_AB")       # A, B
            g2v = gn2g.rearrange("p (c o) -> p c o", o=1)
            b2v = gn2b.rearrange("p (c o) -> p c o", o=1)
            nc.vector.tensor_mul(t1, pc2[:, :, 1:2], g2v)         # f = inv2*g2
            nc.vector.tensor_mul(AB[:, :, 0:1], t1, a_)           # A = f*a
            nc.vector.tensor_sub(t2, d_, pc2[:, :, 0:1])          # d - M2
            nc.vector.tensor_mul(t2, t1, t2)
            nc.vector.tensor_add(AB[:, :, 1:2], t2, b2v)          # B
            for ct in range(CT):
                nc.vector.tensor_scalar(
                    out=dst_hn[:, ct, :], in0=x_sb[:, ct, :],
                    scalar1=AB[:, ct, 0:1], scalar2=AB[:, ct, 1:2],
                    op0=mult, op1=add)
            return ss          # caller applies h = a*x + d when convenient

        _base_depth[0] = len(_stack)
        for _rep in range(repeat):
          try:
            # ---------- per-iteration weight loads (early prefetch) ----------
            p_w = apool(name="p_w", bufs=1)
            qkvwT = p_w.tile([128, CT, 3 * C], fp8)
            dma(out=qkvwT, in_=qkvwT_d)
            sapT = p_w.tile([128, CT, C], fp8)
            dma(out=sapT, in_=sapT_d)
            # CA weights prefetch (fires behind qkvwT on the DMA queues,
            # lands during the exp wall). FFN weights load later (SBUF).
            p_caw = apool(name="p_caw", bufs=1)
            qwT = p_caw.tile([128, CT, C], fp8)
            dma(out=qwT, in_=qwT_d)
            h8 = p_caw.tile([128, CT, L], fp8)   # fp8 shadow of h for DR mms
            capT = p_caw.tile([128, CT, C], fp8)
            dma(out=capT, in_=capT_d)
            p_caa = apool(name="p_caa", bufs=1)
            q2 = p_caa.tile([128, CT, L], bf16)
            ca_out = p_caa.tile([128, CT, L], fp8)

            # ---------- fused GN1+GN2 ----------
            if _rep == 0:
                for ct in range(CT):
                    gn_stats(ct, cold=True)
            if gn2_skip:
                gn_fused(h, None)
                hn = h
                p_hn = None
                h_ss = None
            else:
                p_hn = apool(name="p_hn", bufs=1)
                hn = p_hn.tile([128, CT, L], fp8)
                h_ss = gn_fused(h, hn)

            def apply_h():
                if h_ss is not None:
                    for ct in range(CT):
                        nc.vector.tensor_scalar(
                            out=h[:, ct, :], in0=x_sb[:, ct, :],
                            scalar1=h_ss[:, ct, 0:1], scalar2=h_ss[:, ct, 1:2],
                            op0=mult, op1=add)
            if _rep == 0:
                phase0()
            if stop_after == "gn1":
                apply_h()
                stop_dump(h)

            p_ao = apool(name="p_ao", bufs=1)
            attn_out = p_ao.tile([128, CT, L], fp8)
            p_vaug = apool(name="p_vaug", bufs=1)
            # row stride padded 520->528 so the DoubleRow k-pair step is
            # 16-byte aligned
            VP = 528
            v_aug = p_vaug.tile([128, LT, VP], fp8)
            p_qk = apool(name="p_qk", bufs=1)
            qk = p_qk.tile([128, 2 * CT, L], bf16)      # q tiles 0-3, k tiles 4-7

            p_pt = apool(name="p_pt", bufs=5)

            def sa_scores_alloc(hp):
                return [p_pt.tile([128, LT, L], fp8, tag="PT", bufs=4,
                                  name=f"pt{hp}_{i}") for i in range(2)]

            def sa_scores_mt(hp, mt, pts):
                """S^T then exp for head pair (2hp, 2hp+1), one m-tile."""
                kt_ = 4 + hp
                pp = [psb.tile([128, L], f32, tag="ps", bufs=3,
                               name=f"sps{hp}_{mt}_{i}") for i in range(2)]
                for ch in range(NCH):
                    for i, po in ((0, 0), (1, 64)):
                        nc.tensor.matmul(
                            pp[i][:, ch * CH:(ch + 1) * CH],
                            qk[po:po + 64, kt_, mt * 128:(mt + 1) * 128],
                            qk[po:po + 64, hp, ch * CH:(ch + 1) * CH],
                            start=True, stop=True)
                for i in range(2):
                    nc.scalar.activation(out=pts[i][:, mt, :], in_=pp[i],
                                         func=Exp, scale=SCALE)

            def qkv_group(mt):
                # DoubleRow fp8; weights scaled x8 on host, undone in the
                # bias move (psum * 1/8 + bias)
                ps = psb.tile([128, L], f32, tag="ps", bufs=3, name=f"qkps{mt}")
                for kt in range(0, CT, 2):
                    for ch in range(NCH):
                        nc.tensor.matmul(ps[:, ch * CH:(ch + 1) * CH],
                                         qkvwT[:, kt:kt + 2, mt * 128:(mt + 1) * 128],
                                         hn[:, kt:kt + 2, ch * CH:(ch + 1) * CH],
                                         start=(kt == 0), stop=(kt == CT - 2),
                                         perf_mode=DoubleRow)
                nc.vector.tensor_scalar(out=qk[:, mt, :], in0=ps,
                                        scalar1=0.125, scalar2=qkb[:, mt:mt + 1],
                                        op0=mult, op1=add)

            def v_group(mt):
                # v in transposed (sequence) layout, augmented below
                ps = psb.tile([128, C], f32, tag="ps", bufs=3, name=f"vps{mt}")
                for kt in range(0, CT, 2):
                    nc.tensor.matmul(ps, hn[:, kt:kt + 2, mt * 128:(mt + 1) * 128],
                                     qkvwT[:, kt:kt + 2, 2 * C:3 * C],
                                     start=(kt == 0), stop=(kt == CT - 2),
                                     perf_mode=DoubleRow)
                nc.vector.scalar_tensor_tensor(
                    out=v_aug[:, mt, 0:NH * (HD + 1)].rearrange(
                        "p (h e) -> p h e", e=HD + 1)[:, :, 0:HD],
                    in0=ps.rearrange("p (h e) -> p h e", e=HD),
                    scalar=0.125,
                    in1=vb_bc.rearrange("p (h e) -> p h e", e=HD),
                    op0=mult, op1=add)

            def av_unit(hp, i, ch, pts):
                """AV + softmax-normalize for head 2hp+i, l-chunk ch.
                DoubleRow fp8: two m-tiles contract per matmul."""
                hh = 2 * hp + i
                ps = psb.tile([HD + 1, CH], f32, tag="av", bufs=2,
                              name=f"avps{hh}_{ch}")
                for mt in range(0, LT, 2):
                    nc.tensor.matmul(
                        ps, v_aug[:, mt:mt + 2, hh * (HD + 1):(hh + 1) * (HD + 1)],
                        pts[i][:, mt:mt + 2, ch * CH:(ch + 1) * CH],
                        start=(mt == 0), stop=(mt == LT - 2),
                        perf_mode=DoubleRow)
                rec = scr.tile([1, CH], f32, tag="rec", bufs=3)
                nc.vector.reciprocal(rec, ps[HD:HD + 1, :])
                rb = scr.tile([HD, CH], f32, tag="recb", bufs=3)
                nc.gpsimd.partition_broadcast(rb, rec)
                nc.vector.tensor_mul(
                    out=attn_out[64 * i:64 * i + 64, hp, ch * CH:(ch + 1) * CH],
                    in0=ps[0:HD, :], in1=rb)

            def aux_ones_h():
                nc.vector.tensor_copy(
                    out=v_aug[:, :, 0:NH * (HD + 1)].rearrange(
                        "p m (h e) -> p m h e", e=HD + 1)[:, :, :, HD:HD + 1],
                    in_=ones_t.to_broadcast([128, LT, NH, 1]))
                apply_h()       # h = GN1(x), deferred off the hn critical path

            # The exp wall: 32 score m-tile units, ACT-paced (~2.1us each).
            # PE filler (qkv/v/av units, ~1.2us of slack per slot) is woven in
            # at slot granularity because the PE queue is strict in-order.
            all_pts = [None] * CT
            fill = {0: [lambda: qkv_group(1)], 1: [lambda: qkv_group(5)]}
            for k in range(8):
                fill[2 + k] = [lambda mt=k: v_group(mt)]
            fill[9].append(aux_ones_h)
            fill[10] = [lambda: qkv_group(2)]
            for k in range(4):
                fill[11 + k] = [lambda k=k: av_unit(0, k % 2, k // 2, all_pts[0])]
            fill[15] = [lambda: qkv_group(6)]
            fill[16] = [lambda: qkv_group(3)]
            for k in range(4):
                fill[17 + k] = [lambda k=k: av_unit(1, k % 2, k // 2, all_pts[1])]
            fill[21] = [lambda: qkv_group(7)]
            for k in range(4):
                fill[25 + k] = [lambda k=k: av_unit(2, k % 2, k // 2, all_pts[2])]

            qkv_group(0)
            qkv_group(4)
            for hp in range(CT):
                all_pts[hp] = sa_scores_alloc(hp)
                for mt in range(LT):
                    sa_scores_mt(hp, mt, all_pts[hp])
                    for f in fill.get(hp * 8 + mt, ()):
                        f()
            for k in range(4):
                av_unit(3, k % 2, k // 2, all_pts[3])
            if stop_after == "qkv":
                stop_dump(qk[:, 0:CT, :])
            rel(p_pt)
            rel(p_qk)
            rel(p_vaug)

            # ---------- sa_proj + residual, q2, ch-major pipelined ----------
            def sa_proj(ct, ch):
                ps = psb.tile([128, CH], f32, tag="ps", bufs=3,
                              name=f"sap{ct}_{ch}")
                for kt in range(0, CT, 2):
                    nc.tensor.matmul(ps, sapT[:, kt:kt + 2, ct * 128:(ct + 1) * 128],
                                     attn_out[:, kt:kt + 2, ch * CH:(ch + 1) * CH],
                                     start=(kt == 0), stop=(kt == CT - 2),
                                     perf_mode=DoubleRow)
                nc.vector.scalar_tensor_tensor(
                    out=h[:, ct, ch * CH:(ch + 1) * CH], in0=ps,
                    scalar=sapb[:, ct:ct + 1],
                    in1=h[:, ct, ch * CH:(ch + 1) * CH].bitcast(f32),
                    op0=add, op1=add)
                nc.gpsimd.tensor_copy(
                    out=h8[:, ct, ch * CH:(ch + 1) * CH],
                    in_=h[:, ct, ch * CH:(ch + 1) * CH].bitcast(f32))

            def q2_group(ct, ch):
                ps = psb.tile([128, CH], f32, tag="ps", bufs=3,
                              name=f"q2ps{ct}_{ch}")
                for kt in range(0, CT, 2):
                    nc.tensor.matmul(ps,
                                     qwT[:, kt:kt + 2, ct * 128:(ct + 1) * 128],
                                     h8[:, kt:kt + 2, ch * CH:(ch + 1) * CH],
                                     start=(kt == 0), stop=(kt == CT - 2),
                                     perf_mode=DoubleRow)
                nc.scalar.activation(out=q2[:, ct, ch * CH:(ch + 1) * CH],
                                     in_=ps, func=Ident, scale=0.125,
                                     bias=qb[:, ct:ct + 1])

            # ---------- phase 3: cross-attention (scores/av pipelined) ------

            def ca_scores_ch(hp, ch, p2s):
                """CA scores + exp for head pair hp, one l-chunk (so exps can
                start before the other chunk's q2 exists)."""
                for i, po in ((0, 0), (1, 64)):
                    pp = psb.tile([128, CH], f32, tag="ps", bufs=3,
                                  name=f"cps{hp}_{ch}_{i}")
                    nc.tensor.matmul(pp, k2[po:po + 64, hp, :],
                                     q2[po:po + 64, hp, ch * CH:(ch + 1) * CH],
                                     start=True, stop=True)
                    nc.scalar.activation(out=p2s[i][:, ch * CH:(ch + 1) * CH],
                                         in_=pp, func=Exp, scale=SCALE)

            def ca_av_unit(hp, i, ch, p2s):
                hh = 2 * hp + i
                ps2 = psb.tile([HD + 1, CH], f32, tag="av", bufs=2,
                               name=f"avp2_{hh}_{ch}")
                nc.tensor.matmul(ps2, v2_aug[:, hh * (HD + 1):(hh + 1) * (HD + 1)],
                                 p2s[i][:, ch * CH:(ch + 1) * CH],
                                 start=True, stop=True)
                rec = scr.tile([1, CH], f32, tag="rec", bufs=3)
                nc.vector.reciprocal(rec, ps2[HD:HD + 1, :])
                rb = scr.tile([HD, CH], f32, tag="recb", bufs=3)
                nc.gpsimd.partition_broadcast(rb, rec)
                nc.vector.tensor_mul(
                    out=ca_out[64 * i:64 * i + 64, hp, ch * CH:(ch + 1) * CH],
                    in0=ps2[0:HD, :], in1=rb)

            # sa_proj/q2 ch-major; CA scores+exp per l-chunk start as soon as
            # that chunk's q2 exists; ch0 avs normalize first so ca_proj(ch0)
            # and the FFN can start while ch1 still normalizes.
            for ct in range(CT):
                sa_proj(ct, 0)
            for ct in range(CT):
                q2_group(ct, 0)
            p_ffw = apool(name="p_ffw", bufs=1)
            w1T = p_ffw.tile([128, CT, FF], fp8)
            dma(out=w1T, in_=w1T_d)
            w2T = p_ffw.tile([128, FT, C], fp8)
            dma(out=w2T, in_=w2T_d)
            p_p2 = apool(name="p_p2", bufs=8)
            all_p2 = [[p_p2.tile([128, L], fp8, tag="P2", bufs=8,
                                 name=f"p2_{hp}_{i}") for i in range(2)]
                      for hp in range(CT)]
            for hp in range(CT):
                ca_scores_ch(hp, 0, all_p2[hp])
                sa_proj(hp, 1)          # ct==hp: fill PE while ACT does exps
            for hp in range(CT):
                q2_group(hp, 1)
                ca_scores_ch(hp, 1, all_p2[hp])
                ca_av_unit(hp, 0, 0, all_p2[hp])
                ca_av_unit(hp, 1, 0, all_p2[hp])
            if stop_after == "sa":
                stop_dump(h)

            # ca_proj + residual + x_sb accumulate, then FFN — all ch-major
            p_ff = apool(name="p_ff", bufs=1)
            ff1 = p_ff.tile([128, FT, L], fp8)
            p_of = apool(name="p_of", bufs=2)

            def ca_proj(ct, ch):
                ps = psb.tile([128, CH], f32, tag="ps", bufs=3,
                              name=f"cap{ct}_{ch}")
                for kt in range(0, CT, 2):
                    nc.tensor.matmul(ps, capT[:, kt:kt + 2, ct * 128:(ct + 1) * 128],
                                     ca_out[:, kt:kt + 2, ch * CH:(ch + 1) * CH],
                                     start=(kt == 0), stop=(kt == CT - 2),
                                     perf_mode=DoubleRow)
                nc.vector.scalar_tensor_tensor(
                    out=h[:, ct, ch * CH:(ch + 1) * CH], in0=ps,
                    scalar=capb[:, ct:ct + 1],
                    in1=h[:, ct, ch * CH:(ch + 1) * CH].bitcast(f32),
                    op0=add, op1=add)
                nc.scalar.activation(
                    out=h8[:, ct, ch * CH:(ch + 1) * CH],
                    in_=h[:, ct, ch * CH:(ch + 1) * CH].bitcast(f32),
                    func=Copy)
                nc.gpsimd.tensor_add(x_sb[:, ct, ch * CH:(ch + 1) * CH],
                                     h[:, ct, ch * CH:(ch + 1) * CH].bitcast(f32),
                                     x_sb[:, ct, ch * CH:(ch + 1) * CH])

            def ff1_group(ft, ch):
                ps = psb.tile([128, CH], f32, tag="ps", bufs=3,
                              name=f"f1ps{ft}_{ch}")
                for kt in range(0, CT, 2):
                    nc.tensor.matmul(ps,
                                     w1T[:, kt:kt + 2, ft * 128:(ft + 1) * 128],
                                     h8[:, kt:kt + 2, ch * CH:(ch + 1) * CH],
                                     start=(kt == 0), stop=(kt == CT - 2),
                                     perf_mode=DoubleRow)
                nc.scalar.activation(out=ff1[:, ft, ch * CH:(ch + 1) * CH],
                                     in_=ps, func=Gelu,
                                     bias=b1[:, ft:ft + 1], scale=0.125)

            def ff2_group(ct, ch):
                ps = psb.tile([128, CH], f32, tag="av", bufs=2,
                              name=f"f2ps{ct}_{ch}")
                for kt in range(0, FT, 2):
                    nc.tensor.matmul(ps, w2T[:, kt:kt + 2, ct * 128:(ct + 1) * 128],
                                     ff1[:, kt:kt + 2, ch * CH:(ch + 1) * CH],
                                     start=(kt == 0), stop=(kt == FT - 2),
                                     perf_mode=DoubleRow)
                of = p_of.tile([128, CH], f32, tag="of")
                nc.vector.scalar_tensor_tensor(
                    out=of, in0=ps, scalar=b2[:, ct:ct + 1],
                    in1=x_sb[:, ct, ch * CH:(ch + 1) * CH],
                    op0=add, op1=add)
                dma(out=out_d[:, ct, ch * CH:(ch + 1) * CH], in_=of)

            if stop_after == "ca":
                for hp in range(CT):
                    ca_av_unit(hp, 0, 1, all_p2[hp])
                    ca_av_unit(hp, 1, 1, all_p2[hp])
                for ch in range(NCH):
                    for ct in range(CT):
                        ca_proj(ct, ch)
                stop_dump(h)

            # ch0: ca_proj -> ff1, with ch1's avs woven in (their DVE/Pool
            # normalize chains run under ff1's matmuls)
            for ct in range(CT):
                ca_proj(ct, 0)
            for ft in range(FT):
                ff1_group(ft, 0)
                if ft % 2 == 0 and ft < 16:
                    hp, i = (ft // 2) // 2, (ft // 2) % 2
                    ca_av_unit(hp, i, 1, all_p2[hp])
            for ct in range(CT):
                ca_proj(ct, 1)
            # interleave ff2(ch0) with ff1(ch1); the NEXT rep's GN stats are
            # emitted here so its prefix shrinks (x_sb is final for this rep)
            for ct in range(CT):
                for ft in range(4 * ct, 4 * ct + 4):
                    ff1_group(ft, 1)
                ff2_group(ct, 0)
                gn_stats(ct)
            for ct in range(CT):
                ff2_group(ct, 1)

            rels = [p_of, p_ff, p_p2, p_ffw, p_ao]
            if p_hn is not None:
                rels.append(p_hn)
            rels += [p_caa, p_caw, p_w]
            for p in rels:
                rel(p)
          except _Stop:
            pass
        for p in (p_ctxw, p_kv, psb, scr, small, pers):
            rel(p)

    nc.compile()
    return nc


def _tileK(wT, kt, dt):
    """[K, F] -> [128, kt, F] partition-major layout."""
    K, F = wT.shape
    return np.ascontiguousarray(
        wT.reshape(kt, 128, F).transpose(1, 0, 2)).astype(dt)


def _conv(b):
    """[n] -> [128, n//128] conv-layout bias."""
    return np.ascontiguousarray(np.asarray(b, np.float32).reshape(-1, 128).T)


def prepare_in_maps(inputs):
    import ml_dtypes
    bf = ml_dtypes.bfloat16
    f = lambda a: np.asarray(a, np.float32)
    x = f(inputs["x"]); ctx = f(inputs["context"])

    blob = np.zeros((128, NBLOB), np.float32)

    def setb(name, arr):
        a, b = _BL[name]
        blob[:, a:b] = arr

    setb("gn1g", _conv(inputs["gn_in_g"])); setb("gn1b", _conv(inputs["gn_in_b"]))
    setb("gn2g", _conv(inputs["sa_gn_g"])); setb("gn2b", _conv(inputs["sa_gn_b"]))
    setb("qkb", _conv(f(inputs["qkv_b"])[:2 * C]))
    setb("sapb", _conv(inputs["sa_proj_b"]))
    setb("qb", _conv(inputs["q_b"])); setb("kb", _conv(inputs["k_b"]))
    setb("capb", _conv(inputs["ca_proj_b"]))
    setb("b2", _conv(inputs["b2"])); setb("b1", _conv(inputs["b1"]))
    setb("smask", (np.arange(SP) < S).astype(np.float32).reshape(SP, 1))
    cidx = np.arange(C) // 16
    mask = (cidx[:, None] == np.arange(G)[None, :]).astype(np.float32)  # [C, G]
    setb("mask", mask.reshape(CT, 128, G).transpose(1, 0, 2).reshape(128, CT * G))

    vbrows = np.concatenate([f(inputs["qkv_b"])[2 * C:], f(inputs["v_b"])])
    shared = {
        "blob": blob,
        "gn_maskT": np.ascontiguousarray(mask.T),
        "vbrows": vbrows.reshape(1, 2 * C).copy(),
        "qkv_wT": _tileK(f(inputs["qkv_w"]).T * 8.0, CT, ml_dtypes.float8_e4m3),
        "sa_proj_wT": _tileK(f(inputs["sa_proj_w"]).T, CT, ml_dtypes.float8_e4m3),
        "q_wT": _tileK(f(inputs["q_w"]).T * 8.0, CT, ml_dtypes.float8_e4m3),
        "k_wT": _tileK(f(inputs["k_w"]).T, KTC, bf),
        "v_wT": _tileK(f(inputs["v_w"]).T, KTC, bf),
        "ca_proj_wT": _tileK(f(inputs["ca_proj_w"]).T, CT, ml_dtypes.float8_e4m3),
        "w1T": _tileK(f(inputs["w1"]).T * 8.0, CT, ml_dtypes.float8_e4m3),
        "w2T": _tileK(f(inputs["w2"]).T, FT, ml_dtypes.float8_e4m3),
    }

    in_maps = []
    for b in range(B):
        xb = np.ascontiguousarray(
            x[b].reshape(C, L).reshape(CT, 128, L).transpose(1, 0, 2))
        ctxT = np.zeros((CTX, SP), np.float32)
        ctxT[:, :S] = ctx[b].T
        ctxTb = np.ascontiguousarray(
            ctxT.reshape(KTC, 128, SP).transpose(1, 0, 2)).astype(bf)
        in_maps.append({"x": xb, "ctxT": ctxTb, **shared})
    return in_maps


def kernel(**inputs):
    from concourse.bass_utils import run_bass_kernel_spmd
    if "nc" not in _CACHE:
        _CACHE["nc"] = _build()
    nc = _CACHE["nc"]
    in_maps = prepare_in_maps(inputs)
    res = run_bass_kernel_spmd(nc, in_maps, core_ids=list(range(B)))
    out = np.stack([
        np.ascontiguousarray(res.results[b]["out"].transpose(1, 0, 2)).reshape(C, H, W)
        for b in range(B)])
    return out.astype(np.float32)
